# revision 9
# baseline (speedup 1.0000x reference)
"""Trainium2 Bass kernel for nn_FMNet pixel-shuffle + sigmoid.

reference:  x = FV[:, 64:, :, :]                                 # [B, 64, 64, 64]
            out[b, 8i+r, 8j+c] = sigmoid(x[b, 8r+c, i, j])       # [B, 1, 512, 512]

Per core (4 batches, pure data-parallel over batch):
  - 8 SWDGE loads (gpsimd Q7 generator) of 512 KiB: per (batch, channel-half),
    partition = (b, i2) spatial-row-pair, 512-byte contiguous HBM chunks.
    SWDGE keeps the load descriptor generation off the single shared HWDGE.
  - 8 fused ScalarE ACTIVATE(Sigmoid) ops [128 x 1024] whose strided input AP
    performs the (c', j) -> (j*8 + c') pixel-shuffle interleave in the same
    pass (measured ~2 ns/elem; DVE/GpSimd strided copies are ~4.4 ns/elem).
  - 16 HWDGE stores (SP engine, now otherwise idle) of 256 KiB: per
    (batch, r-quarter), 4 KiB contiguous HBM chunks, issued as soon as the
    two ACTs they depend on are done - keeps the store tail short.
"""

import os
import sys

if "/opt/trn_rl_repo" not in sys.path:
    sys.path.insert(0, "/opt/trn_rl_repo")

import numpy as np

import concourse.bass as bass
from concourse import mybir
from concourse.bass_utils import run_bass_kernel_spmd

N_CORES = 8
B = 32
B_LOC = B // N_CORES   # 4
H = W = 512
S = 64
NG = 8                 # channel groups (r)

LAST_EXEC_NS = None

_cached_nc = None


def _install_trace_hook():
    """Best-effort NTFF hook so BASS_TRACE=1 yields exec_time_ns."""
    try:
        import types

        import antenv

        try:
            from antenv.axon_hooks import get_axon_ntff_profile_hook  # noqa: F401

            return
        except ImportError:
            pass
        mod = types.ModuleType("antenv.axon_hooks")
        _state = {"hook": None}
        mod.set_axon_ntff_profile_hook = lambda h: _state.__setitem__("hook", h)
        mod.get_axon_ntff_profile_hook = lambda: _state["hook"]
        sys.modules["antenv.axon_hooks"] = mod
        antenv.axon_hooks = mod
        from trn_agent_boot.trn_boot import _ntff_profile_via_ctypes

        mod.set_axon_ntff_profile_hook(
            _ntff_profile_via_ctypes("/opt/axon/libaxon_pjrt.so")
        )
    except Exception:
        pass


def _build_nc():
    import contextlib

    F32 = mybir.dt.float32
    nc = bass.Bass("TRN2", num_devices=N_CORES)
    FV = nc.declare_dram_parameter("FV", [B_LOC, 128, S, S], F32, isOutput=False)
    OUT = nc.declare_dram_parameter("OUT", [B_LOC, W, H], F32, isOutput=True)

    # partition p = (b:4, cH:2, i4:16); channel half cH lives in the
    # partition dim so load chunks are 4 spatial rows = 1 KiB
    # TIN free = (c32:32, iq:4, j:64); TOUT free = (iq:4, k:4, q:512)
    tin = nc.alloc_sbuf_tensor("tin", [128, 8192], F32)
    tout = nc.alloc_sbuf_tensor("tout", [128, 8192], F32)

    fv = FV[:]
    out = OUT[:]

    scratch = nc.alloc_sbuf_tensor("scratch", [1, 8], F32)

    def load_aps(b, cH, c_lo, c_hi):
        """(dst, src) APs loading channels c32 in [c_lo, c_hi) of half cH."""
        src = fv[b, 64 + 32 * cH + c_lo : 64 + 32 * cH + c_hi]
        src = src.rearrange("c (i4 iq) j -> i4 c (iq j)", iq=4)  # 1 KiB chunks
        p0 = 32 * b + 16 * cH
        dst = tin.ap()[p0 : p0 + 16, 256 * c_lo : 256 * c_hi]
        return dst, src

    def store_aps(b, cH, kp):
        """(dst, src) APs for batch b, channel-half cH, k-pair kp."""
        # dest rows 32*i4 + 8*iq + 4*cH + 2*kp + k2, cols q
        dst = out[b].rearrange(
            "(i4 iq cH2 kp k2) q -> i4 iq cH2 kp (k2 q)", i4=16, iq=4, cH2=2, kp=2
        )[:, :, cH, kp, :]  # [16, 4, 1024]
        p0 = 32 * b + 16 * cH
        src = tout.ap().rearrange("p (iq kp v) -> p iq kp v", iq=4, kp=2)[
            p0 : p0 + 16, :, kp, :
        ]  # [16, 4, 1024]
        return dst, src

    # load waves by c32 range: w0 = [0,8) (feeds ACT k0), w1 = [8,16), w2 = [16,32)
    WAVES = [(0, 8), (8, 16), (16, 32)]

    with contextlib.ExitStack() as stack:
        block = stack.enter_context(nc.Block())
        sem_w = [stack.enter_context(nc.semaphore(f"sem_w{w}")) for w in range(3)]
        sem_act = stack.enter_context(nc.semaphore("sem_act"))
        sem_out = stack.enter_context(nc.semaphore("sem_out"))

        @block.sync
        def _(sync: bass.BassEngine):
            for w, (c_lo, c_hi) in enumerate(WAVES):
                for b in range(B_LOC):
                    for cH in range(2):
                        dst, src = load_aps(b, cH, c_lo, c_hi)
                        sync.dma_start(out=dst, in_=src).then_inc(sem_w[w], 16)
            for kp in range(2):
                sync.wait_ge(sem_act, 2 * (kp + 1))
                for b in range(B_LOC):
                    for cH in range(2):
                        dst, src = store_aps(b, cH, kp)
                        sync.dma_start(out=dst, in_=src).then_inc(sem_out, 16)
            sync.wait_ge(sem_out, 16 * 16)

        @block.scalar
        def _(scalar: bass.BassEngine):
            # dummy op to pull ACT_TABLE_LOAD (sigmoid) off the critical path
            scalar.activation(
                scratch.ap(), scratch.ap(), mybir.ActivationFunctionType.Sigmoid
            )
            for k in range(4):
                if k < 3:
                    scalar.wait_ge(sem_w[k], 16 * 8)
                # in: (iq, j, c') strided read; c32 = 8k + c'
                tin_v = tin.ap().rearrange(
                    "p (kk cc iq j) -> p kk iq j cc", kk=4, cc=8, iq=4
                )[:, k, :, :, :]  # [128, 4, 64, 8]
                # out: (iq, [k], q) with q = j*8+c' contiguous
                tout_v = tout.ap().rearrange(
                    "p (iq kk q) -> p iq kk q", iq=4, kk=4
                )[:, :, k, :]  # [128, 4, 512]
                scalar.activation(
                    tout_v, tin_v, mybir.ActivationFunctionType.Sigmoid
                ).then_inc(sem_act, 1)

    return nc


def kernel(FV, batch_size=None, W=None, H=None, **_ignored):
    global _cached_nc, LAST_EXEC_NS
    FV = np.asarray(FV, dtype=np.float32)
    assert FV.shape == (B, 128, S, S), FV.shape

    trace = bool(os.environ.get("BASS_TRACE"))
    if trace:
        _install_trace_hook()

    if _cached_nc is None:
        _cached_nc = _build_nc()
    nc = _cached_nc

    in_maps = [{"FV": FV[k * B_LOC : (k + 1) * B_LOC]} for k in range(N_CORES)]
    res = run_bass_kernel_spmd(nc, in_maps, list(range(N_CORES)), trace=trace)
    if trace:
        LAST_EXEC_NS = res.exec_time_ns

    outs = [res.results[k]["OUT"] for k in range(N_CORES)]
    full = np.concatenate(outs, axis=0)  # [32, 512, 512]
    return full[:, None, :, :].astype(np.float32)


# revision 11
# speedup vs baseline: 1.6873x; 1.6873x over previous
"""Trainium2 Bass kernel for nn_FMNet pixel-shuffle + sigmoid.

reference:  x = FV[:, 64:, :, :]                                 # [B, 64, 64, 64]
            out[b, 8i+r, 8j+c] = sigmoid(x[b, 8r+c, i, j])       # [B, 1, 512, 512]

Per core (4 batches, pure data-parallel over batch):
  - 8 SWDGE loads (gpsimd Q7 generator) of 512 KiB: per (batch, channel-half),
    partition = (b, i2) spatial-row-pair, 512-byte contiguous HBM chunks.
    SWDGE keeps the load descriptor generation off the single shared HWDGE.
  - 8 fused ScalarE ACTIVATE(Sigmoid) ops [128 x 1024] whose strided input AP
    performs the (c', j) -> (j*8 + c') pixel-shuffle interleave in the same
    pass (measured ~2 ns/elem; DVE/GpSimd strided copies are ~4.4 ns/elem).
  - 16 HWDGE stores (SP engine, now otherwise idle) of 256 KiB: per
    (batch, r-quarter), 4 KiB contiguous HBM chunks, issued as soon as the
    two ACTs they depend on are done - keeps the store tail short.
"""

import os
import sys

if "/opt/trn_rl_repo" not in sys.path:
    sys.path.insert(0, "/opt/trn_rl_repo")

import numpy as np

import concourse.bass as bass
from concourse import mybir
from concourse.bass_utils import run_bass_kernel_spmd

N_CORES = 8
B = 32
B_LOC = B // N_CORES   # 4
H = W = 512
S = 64
NG = 8                 # channel groups (r)

LAST_EXEC_NS = None

_cached_nc = None


def _install_trace_hook():
    """Best-effort NTFF hook so BASS_TRACE=1 yields exec_time_ns."""
    try:
        import types

        import antenv

        try:
            from antenv.axon_hooks import get_axon_ntff_profile_hook  # noqa: F401

            return
        except ImportError:
            pass
        mod = types.ModuleType("antenv.axon_hooks")
        _state = {"hook": None}
        mod.set_axon_ntff_profile_hook = lambda h: _state.__setitem__("hook", h)
        mod.get_axon_ntff_profile_hook = lambda: _state["hook"]
        sys.modules["antenv.axon_hooks"] = mod
        antenv.axon_hooks = mod
        from trn_agent_boot.trn_boot import _ntff_profile_via_ctypes

        mod.set_axon_ntff_profile_hook(
            _ntff_profile_via_ctypes("/opt/axon/libaxon_pjrt.so")
        )
    except Exception:
        pass


def _build_nc():
    import contextlib

    F32 = mybir.dt.float32
    nc = bass.Bass("TRN2", num_devices=N_CORES)
    FV = nc.declare_dram_parameter("FV", [B_LOC, 128, S, S], F32, isOutput=False)
    OUT = nc.declare_dram_parameter("OUT", [B_LOC, W, H], F32, isOutput=True)

    # partition p = (b:4, i2:32); TIN_h free = (c32:32, ip, j) for channel
    # half h; TOUT_h free = (ip:2, r4:4, q:512) for r half h
    tin = [nc.alloc_sbuf_tensor(f"tin{h}", [128, 4096], F32) for h in range(2)]
    tout = [nc.alloc_sbuf_tensor(f"tout{h}", [128, 4096], F32) for h in range(2)]

    fv = FV[:]
    out = OUT[:]

    scratch = nc.alloc_sbuf_tensor("scratch", [1, 8], F32)

    def load_aps(b, g):
        """(dst, src) APs loading channel octant g of batch b (512 B chunks)."""
        h, g4 = divmod(g, 4)
        src = fv[b, 64 + 8 * g : 64 + 8 * g + 8]  # [8, 64, 64]
        src = src.rearrange("c (i2 ip) j -> i2 c (ip j)", ip=2)
        dst = tin[h].ap()[32 * b : 32 * b + 32, 1024 * g4 : 1024 * (g4 + 1)]
        return dst, src

    def store_aps(b, rq):
        """(dst, src) APs for the store of batch b, r-quarter rq."""
        h, k = divmod(rq, 2)  # tout half h, quarter k within half
        # dest rows 16*i2 + 8*ip + (2rq + r2), cols q
        dst = out[b].rearrange(
            "(i2 ip rr r2) q -> i2 ip rr (r2 q)", i2=32, ip=2, rr=4
        )[:, :, rq, :]  # [32, 2, 1024]
        src = tout[h].ap().rearrange(
            "p (ip r2 v) -> p ip r2 v", ip=2, r2=2
        )[32 * b : 32 * b + 32, :, k, :]  # [32, 2, 1024]
        return dst, src

    with contextlib.ExitStack() as stack:
        block = stack.enter_context(nc.Block())
        sem_oct = [stack.enter_context(nc.semaphore(f"sem_o{g}")) for g in range(NG)]
        sem_act = stack.enter_context(nc.semaphore("sem_act"))
        sem_out = stack.enter_context(nc.semaphore("sem_out"))

        # Two concurrent load streams: ring-queued DMAs serialize per ring and
        # run at (partitions/8) x 27 GB/s, so b0/b1 (SBUF ports 0-63, even
        # SDMA engines) go via SP HWDGE while b2/b3 (odd engines) go via the
        # GpSimd SWDGE generator in parallel.
        @block.sync
        def _(sync: bass.BassEngine):
            for g in range(NG):
                for b in (0, 1):
                    dst, src = load_aps(b, g)
                    sync.dma_start(out=dst, in_=src).then_inc(sem_oct[g], 16)
            for rq in range(4):
                sync.wait_ge(sem_act, 2 * (rq + 1))
                for b in range(B_LOC) if rq < 3 else (0, 1):
                    dst, src = store_aps(b, rq)
                    sync.dma_start(out=dst, in_=src).then_inc(sem_out, 16)
            sync.wait_ge(sem_out, 16 * 16)

        @block.gpsimd
        def _(g_eng: bass.BassEngine):
            for g in range(NG):
                for b in (2, 3):
                    dst, src = load_aps(b, g)
                    g_eng.dma_start(out=dst, in_=src).then_inc(sem_oct[g], 16)

        @block.scalar
        def _(scalar: bass.BassEngine):
            # dummy op to pull ACT_TABLE_LOAD (sigmoid) off the critical path
            scalar.activation(
                scratch.ap(), scratch.ap(), mybir.ActivationFunctionType.Sigmoid
            )
            for r in range(NG):
                h, r4 = divmod(r, 4)
                scalar.wait_ge(sem_oct[r], 16 * B_LOC)
                # in: (ip, j, c') strided read of the (c', ip, j) tile slice
                tin_v = (
                    tin[h]
                    .ap()[:, 1024 * r4 : 1024 * (r4 + 1)]
                    .rearrange("p (c ip j) -> p ip j c", c=8, ip=2)
                )
                # out: (ip, [r4], q) with q = j*8+c' contiguous
                tout_v = tout[h].ap().rearrange(
                    "p (ip r4 q) -> p ip r4 q", ip=2, r4=4
                )[:, :, r4, :]
                scalar.activation(
                    tout_v, tin_v, mybir.ActivationFunctionType.Sigmoid
                ).then_inc(sem_act, 1)
            # tail stores for b2/b3 of the last r-quarter on the ACT HWDGE
            # ring (free once the ACTIVATEs are done)
            scalar.wait_ge(sem_act, NG)
            for b in (2, 3):
                dst, src = store_aps(b, 3)
                scalar.dma_start(out=dst, in_=src).then_inc(sem_out, 16)

    return nc


def kernel(FV, batch_size=None, W=None, H=None, **_ignored):
    global _cached_nc, LAST_EXEC_NS
    FV = np.asarray(FV, dtype=np.float32)
    assert FV.shape == (B, 128, S, S), FV.shape

    trace = bool(os.environ.get("BASS_TRACE"))
    if trace:
        _install_trace_hook()

    if _cached_nc is None:
        _cached_nc = _build_nc()
    nc = _cached_nc

    in_maps = [{"FV": FV[k * B_LOC : (k + 1) * B_LOC]} for k in range(N_CORES)]
    res = run_bass_kernel_spmd(nc, in_maps, list(range(N_CORES)), trace=trace)
    if trace:
        LAST_EXEC_NS = res.exec_time_ns

    outs = [res.results[k]["OUT"] for k in range(N_CORES)]
    full = np.concatenate(outs, axis=0)  # [32, 512, 512]
    return full[:, None, :, :].astype(np.float32)
